# revision 30
# baseline (speedup 1.0000x reference)
"""AttentionMILPooling Trainium2 kernel.

Math (matches the jax reference):
    scores  = tanh(X @ W1 + b1) @ W2 + b2          # [T, 1]
    weights = softmax(scores, axis=0)              # global over all T
    out[b]  = sum_{i in bag b} weights[i] * X[i]   # [64, 512]

Key identities used:
  * b2 cancels exactly in the softmax, so it is dropped.
  * scores are bounded (|s| <= sum|W2| ~ 13) so no max-subtraction is
    needed; exp stays well inside fp32 range.
  * out[b] = (sum_{i in b} exp(s_i) * X_i) / Z with Z = sum_i exp(s_i):
    each core computes unnormalized per-bag sums U and returns all its
    per-row exp values; the host sums Z globally and divides once.

Structure: the host pre-casts X to bf16 and ALSO passes a pre-transposed
copy X^T (total DRAM traffic unchanged vs one fp32 copy), which removes
the entire PE transpose pass and all PSUM->SBUF relayout copies of the
earlier design.  The per-bag weighted sums are computed with X tiles as
the *stationary* operand and the exp-weight column as a 1-column moving
operand (output in [feature, bag] layout, untransposed on the host), so
their PE cost is stationary-load-bound instead of 512-column streams.
The kernel is DMA-bound (all 16 queues ~100% busy streaming 33.6MB/core
at ~392GB/s aggregate).  DMA blocks are 2048 rows -- the largest that
keeps a whole block inside one bag -- giving 16KB (xb) / 4KB (xt)
contiguous runs per partition, which measured ~13% faster per queue than
1024-row blocks.  The first and last blocks' X^T arrive as four
independent per-group quarter-tiles so the pipeline ramps as soon as
512KB lands and the final PE chunk after the stream drains is small;
outputs stream out under the drain.  Blocks 1-2 skip their X^T DMA
entirely: their X^T tiles are rebuilt on-device by PE transposes of the
already-loaded x tiles (+4.2us idle-PE work per block buys 5.1us of
saturated stream time each) -- the measured PE/DMA balance point.
Measured: 111.4-112.0us HW exec (156.3us session baseline).

Row permutation: within each 2048-row block, partition p holds rows
{16p+q}. X^T is passed with columns in the same permuted order
(host-side permute), so scores, exp-weights and U tiles all index rows
identically.  All math is row-order-free within a bag (bag_rows % 2048
== 0 keeps whole blocks inside one bag).

Per-core device pipeline, per 512-row group g (32 groups; block = 4
groups, DMA'd as one 2MB xb load [128,16,512] + one 2MB xt load
[128,4,2048]):
  PE  : 8x matmul H^T = W1^T @ X^T (accum 4 f-chunks, 2 hid-chunks)
  ACT : 2x tanh(H^T + b1) -> bf16 (b1 as per-partition bias)
  PE  : 8x matmul s = tanhH^T.T @ W2 (1-col moving, accum hid-chunks)
  ACT : 1x exp(s) -> wsave column (bf16)
  PE  : 16x matmul U^T[bag] += X_chunk^T @ w_col (X stationary, 1-col)
Emission is software-pipelined (s for g-1, U for g-2) so the in-order PE
stream never waits on same-group ACT results.  Per-bag U^T accumulates
in a [128, 4] PSUM tile over the bag's 16 tiles (start=True on the first
matmul marks the whole 2KB zero region pending-zero, so the other
columns write clean), then is copied to SBUF and DMA'd out at the end
([feature, bag] layout; host transposes).
"""

import numpy as np
import ml_dtypes

N_CORES = 8
F = 512  # feature dim
HID = 256  # hidden dim
P = 128  # partitions
BLK = 2048  # rows per DMA block
JT = 4  # 128-row tiles per processing group
GR = JT * P  # rows per processing group (512)

_COMPILED_CACHE = {}


def _build_program(n_tiles, tile_col, n_cols):
    """Build the SPMD bass program.

    n_tiles: number of 128-row tiles per core (must be divisible by 8).
    tile_col: list, local bag-column index for each tile (same on all cores).
    n_cols: number of local bag columns.
    """
    import concourse.bacc as bacc
    import concourse.mybir as mybir
    from concourse.tile import TileContext
    from concourse.masks import make_identity

    f32 = mybir.dt.float32
    bf16 = mybir.dt.bfloat16
    FC = F // P  # 4 feature chunks
    MC = HID // P  # 2 hidden chunks
    rows_per_core = n_tiles * P
    n_groups = n_tiles // JT
    n_blocks = rows_per_core // BLK
    GPB = BLK // GR  # groups per block
    LOOKAHEAD = 2  # blocks of DMA prefetch

    nc = bacc.Bacc(
        "TRN2", target_bir_lowering=False, debug=False, num_devices=N_CORES
    )

    x = nc.declare_dram_parameter("x", [rows_per_core, F], bf16, isOutput=False)
    xt = nc.declare_dram_parameter("xt", [F, rows_per_core], bf16, isOutput=False)
    w1 = nc.declare_dram_parameter("w1", [P, FC, MC, P], bf16, isOutput=False)
    b1 = nc.declare_dram_parameter("b1", [P, MC], f32, isOutput=False)
    w2 = nc.declare_dram_parameter("w2", [P, MC, 1], bf16, isOutput=False)
    u_out = nc.declare_dram_parameter("u", [P, FC, n_cols], f32, isOutput=True)
    w_out = nc.declare_dram_parameter("w", [P, n_tiles], bf16, isOutput=True)

    with TileContext(nc) as tc:
        with (
            tc.tile_pool(name="const", bufs=1) as const_pool,
            tc.tile_pool(name="xb", bufs=4) as xb_pool,
            tc.tile_pool(name="xt", bufs=3) as xt_pool,
            tc.tile_pool(name="xte", bufs=8) as xte_pool,
            tc.tile_pool(name="th", bufs=3) as th_pool,
            tc.tile_pool(name="out_sb", bufs=1) as out_pool,
            tc.tile_pool(name="hp", bufs=3, space="PSUM") as hp_pool,
            tc.tile_pool(name="sp", bufs=2, space="PSUM") as sp_pool,
            tc.tile_pool(name="acc", bufs=1, space="PSUM") as acc_pool,
            tc.tile_pool(name="pt", bufs=2, space="PSUM") as pt_pool,
        ):
            xb_hist = {}
            xt_hist = {}
            xt_parts = {}
            # this block's X^T is built on-device by PE transposes of its
            # xb tile (its 2MB xt DMA is skipped): trades idle PE/DVE for
            # DMA stream time on a DMA-bound kernel.
            tr_blocks = {1, 2} if n_blocks > 4 else set()
            tr_groups = {
                g
                for tb in tr_blocks
                for g in range(tb * GPB, (tb + 1) * GPB)
            }

            def emit_load(bb, split_xt=False):
                # X^T block first (it gates the H matmuls): partition p
                # holds feature rows {128c + p}, 1024 columns (2KB x 4
                # runs per partition).  Block 0 is loaded as two halves
                # so the first H matmuls start sooner.
                if bb in tr_blocks:
                    pass  # X^T comes from emit_transpose_group
                elif split_xt:
                    for hh in range(GPB):
                        xth = xte_pool.tile(
                            [P, FC, GR], bf16, name="xth", tag="xth"
                        )
                        xt_parts[bb * GPB + hh] = xth
                        nc.sync.dma_start(
                            out=xth,
                            in_=xt[
                                :, bb * BLK + hh * GR : bb * BLK + (hh + 1) * GR
                            ].rearrange("(c p) i -> p c i", p=P),
                        )
                else:
                    xtt = xt_pool.tile([P, FC, BLK], bf16, name="xt", tag="xt")
                    xt_hist[bb] = xtt
                    nc.sync.dma_start(
                        out=xtt,
                        in_=xt[:, bb * BLK : (bb + 1) * BLK].rearrange(
                            "(c p) i -> p c i", p=P
                        ),
                    )
                # partition p holds rows {8p+q} of the block: 8KB
                # contiguous per partition in bf16.
                xbt = xb_pool.tile([P, BLK // P, F], bf16, name="xb", tag="xb")
                xb_hist[bb] = xbt
                nc.sync.dma_start(
                    out=xbt,
                    in_=x[bb * BLK : (bb + 1) * BLK, :].rearrange(
                        "(p q) f -> p q f", p=P
                    ),
                )

            # ---- constants (host pre-chunked, no on-device relayout) ----
            # w1b[p, c, m, j] = W1[c*128+p, m*128+j], bf16
            w1b = const_pool.tile([P, FC, MC, P], bf16)
            nc.sync.dma_start(out=w1b, in_=w1[:, :, :, :])
            # w2b[p, m, 0] = W2[m*128+p, 0], bf16
            w2b = const_pool.tile([P, MC, 1], bf16)
            nc.sync.dma_start(out=w2b, in_=w2[:, :, :])
            # b1s[p, m] = b1[m*128+p], fp32 (per-partition tanh bias)
            b1s = const_pool.tile([P, MC], f32)
            nc.sync.dma_start(out=b1s, in_=b1[:, :])

            emit_load(0, split_xt=True)
            for bb in range(1, min(LOOKAHEAD + 1, n_blocks)):
                emit_load(bb, split_xt=(bb == n_blocks - 1))

            ident = const_pool.tile([P, P], bf16)
            make_identity(nc, ident)

            # PE_HAM pre-warm: the PE clock sits at 1.2GHz until ~3.4us of
            # sustained activity (first 9 H matmuls measured 427-630ns vs
            # 216ns warm).  PE is idle from ~6us (ident ready) to ~14us
            # (first X^T quarter + weights land), so burn that window on
            # dummy ident matmuls to enter the real work at full clock.
            warm = hp_pool.tile([P, GR], f32, name="hp", tag="hp")
            for _ in range(100):
                nc.tensor.matmul(
                    warm[:, 0:P], ident, ident, start=True, stop=True
                )

            def emit_transpose_group(gg):
                # X^T tile for one 512-row group from xb chunks: 16 PE
                # transposes -> PSUM (2 halves), 4 DVE copies -> SBUF,
                # emitted 2 groups ahead of its H matmuls.
                tb, th_ = divmod(gg, GPB)
                xb_g = xb_hist[tb]
                xth = xte_pool.tile([P, FC, GR], bf16, name="xth", tag="xth")
                xt_parts[gg] = xth
                for half in range(2):
                    pt = pt_pool.tile([P, 2, FC, P], bf16, name="pt", tag="pt")
                    for jj in range(2):
                        q = JT * th_ + 2 * half + jj
                        for c in range(FC):
                            nc.tensor.matmul(
                                pt[:, jj, c, :],
                                xb_g[:, q, c * P : (c + 1) * P],
                                ident,
                                is_transpose=True,
                                start=(jj == 0 and c == 0),
                                stop=(jj == 1 and c == FC - 1),
                            )
                    for jj in range(2):
                        j = 2 * half + jj
                        nc.vector.tensor_copy(
                            out=xth[:, :, j * P : (j + 1) * P],
                            in_=pt[:, jj],
                        )

            # softmax weights for every row, bf16 (also read back by the
            # host to form the global softmax denominator)
            wsave = const_pool.tile([P, n_tiles], bf16)

            # U^T accumulators: one [128, FC] PSUM tile per bag while its
            # 16 tiles accumulate; copied into u_sb at bag end.
            u_sb = out_pool.tile([P, FC, n_cols], f32)
            first_tile = {}
            last_tile = {}
            for t, cl in enumerate(tile_col):
                first_tile.setdefault(cl, t)
                last_tile[cl] = t
            u_bag = [None] * n_cols

            th_hist = {}
            sp_hist = {}

            def emit_s(gg):
                th_g = th_hist[gg]
                sp = sp_pool.tile([P, JT], f32, name="sp", tag="sp")
                sp_hist[gg] = sp
                for j in range(JT):
                    for m in range(MC):
                        nc.tensor.matmul(
                            sp[:, j : j + 1],
                            th_g[:, m, j, :],
                            w2b[:, m, :],
                            start=(j == 0 and m == 0),
                            stop=(j == JT - 1 and m == MC - 1),
                        )
                nc.scalar.activation(
                    wsave[:, gg * JT : (gg + 1) * JT],
                    sp,
                    mybir.ActivationFunctionType.Exp,
                )
                del sp_hist[gg]
                del th_hist[gg]

            def emit_u(gg):
                bb, h = divmod(gg, GPB)
                xb_g = xb_hist[bb]
                for j in range(JT):
                    t = JT * gg + j
                    q = JT * h + j
                    col = tile_col[t]
                    if u_bag[col] is None:
                        u_bag[col] = acc_pool.tile(
                            [P, FC], f32, name="u_bag", tag="u_bag"
                        )
                    for c in range(FC):
                        # start=True marks the whole 2KB zero region
                        # pending-zero, so only the bag's first matmul
                        # starts; c=1..3 of the first tile then write
                        # (not accumulate) their still-pending columns.
                        nc.tensor.matmul(
                            u_bag[col][:, c : c + 1],
                            xb_g[:, q, c * P : (c + 1) * P],
                            wsave[:, t : t + 1],
                            start=(t == first_tile[col] and c == 0),
                            stop=(t == last_tile[col] and c == FC - 1),
                        )
                    if t == last_tile[col]:
                        nc.vector.tensor_copy(out=u_sb[:, :, col], in_=u_bag[col])
                        u_bag[col] = None
                if h == GPB - 1:
                    del xb_hist[bb]

            # ---- main loop over 512-row groups (software-pipelined) ----
            for g in range(n_groups):
                bb, h = divmod(g, GPB)
                if h == 0 and (bb + LOOKAHEAD) < n_blocks and (
                    bb + LOOKAHEAD
                ) not in xb_hist:
                    emit_load(
                        bb + LOOKAHEAD,
                        split_xt=(bb + LOOKAHEAD == n_blocks - 1),
                    )
                edge = g in xt_parts
                xtb = xt_parts[g] if edge else xt_hist[bb]

                # H^T[m*128+p, r] over the group's 512 rows, accumulating
                # feature chunks; then tanh(H^T + b1) -> bf16 per m-chunk.
                # c-inner (same-PSUM-bank consecutive) order: measured
                # faster than m-alternating, whose bank switch exposes the
                # next stationary load.
                th = th_pool.tile([P, MC, JT, P], bf16)
                th_hist[g] = th
                for m in range(MC):
                    hp = hp_pool.tile([P, GR], f32, name="hp", tag="hp")
                    for c in range(FC):
                        nc.tensor.matmul(
                            hp,
                            w1b[:, c, m, :],
                            (xtb[:, c, :] if edge else
                             xtb[:, c, h * GR : (h + 1) * GR]),
                            start=(c == 0),
                            stop=(c == FC - 1),
                        )
                    nc.scalar.activation(
                        th[:, m],
                        hp.rearrange("p (j r) -> p j r", j=JT),
                        mybir.ActivationFunctionType.Tanh,
                        bias=b1s[:, m : m + 1],
                    )
                if edge:
                    del xt_parts[g]
                elif h == GPB - 1:
                    del xt_hist[bb]
                if (g + 2) in tr_groups:
                    emit_transpose_group(g + 2)

                # pipelined: scores for g-1, bag accumulation for g-2
                if g >= 1:
                    emit_s(g - 1)
                if g >= 2:
                    emit_u(g - 2)

            # drain: scores for the last group first so its exp (ACT)
            # overlaps the U matmuls of g-2 on PE
            emit_s(n_groups - 1)
            emit_u(n_groups - 2)
            # exp weights are complete: stream them out under the last Us,
            # along with every bag but the last (already copied to SBUF)
            nc.sync.dma_start(out=w_out[:, :], in_=wsave)
            if n_cols > 1:
                nc.sync.dma_start(
                    out=u_out[:, :, : n_cols - 1], in_=u_sb[:, :, : n_cols - 1]
                )
            emit_u(n_groups - 1)

            # ---- epilogue: DMA the last bag's U^T out ----
            nc.sync.dma_start(
                out=u_out[:, :, n_cols - 1 :], in_=u_sb[:, :, n_cols - 1 :]
            )

    nc.compile()
    return nc


def _host_prep(X_core):
    """Cast the core's X shard to bf16 and build the permuted X^T copy.

    xb: natural [rows, F] bf16 (the device DMA applies the {8p+q} row
        permutation via its access pattern).
    xt: [F, rows] bf16 with columns permuted so that block bb's columns
        are ordered (q, p) -> row bb*1024 + 8p + q, matching xb's layout.
    """
    rows = X_core.shape[0]
    xb = np.ascontiguousarray(X_core).astype(ml_dtypes.bfloat16)
    xt = (
        X_core.reshape(rows // BLK, P, BLK // P, F)
        .transpose(3, 0, 2, 1)
        .reshape(F, rows)
        .astype(ml_dtypes.bfloat16)
    )
    return xb, np.ascontiguousarray(xt)


def _run_device(X, W1, b1, W2, bag_rows, trace=False, trace_kwargs=None):
    from concourse.bass_utils import run_bass_kernel_spmd

    rows_per_core = X.shape[0] // N_CORES
    n_tiles = rows_per_core // P
    tiles_per_bag = bag_rows // P
    n_cols = n_tiles // tiles_per_bag
    tile_col = [t // tiles_per_bag for t in range(n_tiles)]

    key = (rows_per_core, bag_rows)
    if key in _COMPILED_CACHE:
        nc = _COMPILED_CACHE[key]
    else:
        nc = _build_program(n_tiles, tile_col, n_cols)
        _COMPILED_CACHE[key] = nc

    FC = F // P
    MC = HID // P
    w1b = np.ascontiguousarray(
        np.asarray(W1, np.float32).reshape(FC, P, MC, P).transpose(1, 0, 2, 3)
    ).astype(ml_dtypes.bfloat16)
    w2b = np.ascontiguousarray(
        np.asarray(W2, np.float32).reshape(MC, P, 1).transpose(1, 0, 2)
    ).astype(ml_dtypes.bfloat16)
    b1s = np.ascontiguousarray(
        np.asarray(b1, np.float32).reshape(MC, P).T, np.float32
    )

    in_maps = []
    for c in range(N_CORES):
        xb_c, xt_c = _host_prep(
            np.asarray(
                X[c * rows_per_core : (c + 1) * rows_per_core], np.float32
            )
        )
        in_maps.append(
            {"x": xb_c, "xt": xt_c, "w1": w1b, "b1": b1s, "w2": w2b}
        )
    kw = dict(trace_kwargs or {})
    res = run_bass_kernel_spmd(
        nc, in_maps, list(range(N_CORES)), trace=trace, **kw
    )

    U = np.zeros((N_CORES * n_cols, F), np.float32)
    Z = np.float64(0.0)
    for c in range(N_CORES):
        # u[p, fc, col] -> U[col, fc*128 + p]
        U[c * n_cols : (c + 1) * n_cols] = (
            np.asarray(res.results[c]["u"]).transpose(2, 1, 0).reshape(n_cols, F)
        )
        Z += np.asarray(res.results[c]["w"], np.float64).sum()
    return U, Z, res


def _kernel_numpy(instance_features, bag_sizes, W1, b1, W2, b2):
    """Exact-math fallback for bag layouts the device program doesn't cover."""
    X = np.asarray(instance_features, np.float32)
    s = np.tanh(X @ W1 + b1) @ W2.reshape(-1, 1) + np.asarray(b2).reshape(1, -1)
    s = s - s.max()
    w = np.exp(s)
    w = w / w.sum()
    offsets = np.cumsum(np.asarray(bag_sizes, np.int64))
    seg = np.searchsorted(offsets, np.arange(X.shape[0]), side="right")
    out = np.zeros((len(bag_sizes), X.shape[1]), np.float32)
    np.add.at(out, seg[seg < len(bag_sizes)], (X * w)[seg < len(bag_sizes)])
    return out


def kernel(**inputs):
    X = np.asarray(inputs["instance_features"], np.float32)
    bag_sizes = np.asarray(inputs["bag_sizes"], np.int64)
    W1 = np.asarray(inputs["W1"], np.float32)
    b1 = np.asarray(inputs["b1"], np.float32)
    W2 = np.asarray(inputs["W2"], np.float32)
    b2 = np.asarray(inputs["b2"], np.float32)

    T, Fdim = X.shape
    B = bag_sizes.shape[0]
    bag = int(bag_sizes[0]) if B else 0
    # Device path constraints: equal whole bags per core, 1024-row DMA
    # blocks, and the row permutation needs bag_rows % 1024 == 0.
    aligned = (
        Fdim == F
        and B > 0
        and np.all(bag_sizes == bag)
        and bag % BLK == 0
        and bag * B == T
        and T % N_CORES == 0
        and (T // N_CORES) % BLK == 0
        and (T // N_CORES) % bag == 0
    )
    if not aligned:
        return _kernel_numpy(X, bag_sizes, W1, b1, W2, b2)

    U, Z, _ = _run_device(X, W1, b1, W2, bag)
    return (U / np.float32(Z)).astype(np.float32)


# revision 31
# speedup vs baseline: 1.2447x; 1.2447x over previous
"""AttentionMILPooling Trainium2 kernel.

Math (matches the jax reference):
    scores  = tanh(X @ W1 + b1) @ W2 + b2          # [T, 1]
    weights = softmax(scores, axis=0)              # global over all T
    out[b]  = sum_{i in bag b} weights[i] * X[i]   # [64, 512]

Key identities used:
  * b2 cancels exactly in the softmax, so it is dropped.
  * scores are bounded (|s| <= sum|W2| ~ 13) so no max-subtraction is
    needed; exp stays well inside fp32 range.
  * out[b] = (sum_{i in b} exp(s_i) * X_i) / Z with Z = sum_i exp(s_i):
    each core computes unnormalized per-bag sums U and returns all its
    per-row exp values; the host sums Z globally and divides once.

Structure: the host pre-casts X to bf16 and ALSO passes a pre-transposed
copy X^T (total DRAM traffic unchanged vs one fp32 copy), which removes
the entire PE transpose pass and all PSUM->SBUF relayout copies of the
earlier design.  The per-bag weighted sums are computed with X tiles as
the *stationary* operand and the exp-weight column as a 1-column moving
operand (output in [feature, bag] layout, untransposed on the host), so
their PE cost is stationary-load-bound instead of 512-column streams.
The kernel is DMA-bound (all 16 queues ~100% busy streaming 33.6MB/core
at ~392GB/s aggregate).  DMA blocks are 2048 rows -- the largest that
keeps a whole block inside one bag -- giving 16KB (xb) / 4KB (xt)
contiguous runs per partition, which measured ~13% faster per queue than
1024-row blocks.  The first and last blocks' X^T arrive as four
independent per-group quarter-tiles so the pipeline ramps as soon as
512KB lands and the final PE chunk after the stream drains is small;
outputs stream out under the drain.  Blocks 1-2 skip their X^T DMA
entirely: their X^T tiles are rebuilt on-device by PE transposes of the
already-loaded x tiles (+4.2us idle-PE work per block buys 5.1us of
saturated stream time each) -- the measured PE/DMA balance point.
Measured: 111.4-112.0us HW exec (156.3us session baseline).

Row permutation: within each 2048-row block, partition p holds rows
{16p+q}. X^T is passed with columns in the same permuted order
(host-side permute), so scores, exp-weights and U tiles all index rows
identically.  All math is row-order-free within a bag (bag_rows % 2048
== 0 keeps whole blocks inside one bag).

Per-core device pipeline, per 512-row group g (32 groups; block = 4
groups, DMA'd as one 2MB xb load [128,16,512] + one 2MB xt load
[128,4,2048]):
  PE  : 8x matmul H^T = W1^T @ X^T (accum 4 f-chunks, 2 hid-chunks)
  ACT : 2x tanh(H^T + b1) -> bf16 (b1 as per-partition bias)
  PE  : 8x matmul s = tanhH^T.T @ W2 (1-col moving, accum hid-chunks)
  ACT : 1x exp(s) -> wsave column (bf16)
  PE  : 16x matmul U^T[bag] += X_chunk^T @ w_col (X stationary, 1-col)
Emission is software-pipelined (s for g-1, U for g-2) so the in-order PE
stream never waits on same-group ACT results.  Per-bag U^T accumulates
in a [128, 4] PSUM tile over the bag's 16 tiles (start=True on the first
matmul marks the whole 2KB zero region pending-zero, so the other
columns write clean), then is copied to SBUF and DMA'd out at the end
([feature, bag] layout; host transposes).
"""

import numpy as np
import ml_dtypes

N_CORES = 8
F = 512  # feature dim
HID = 256  # hidden dim
P = 128  # partitions
BLK = 2048  # rows per DMA block
JT = 4  # 128-row tiles per processing group
GR = JT * P  # rows per processing group (512)

_COMPILED_CACHE = {}


def _build_program(n_tiles, tile_col, n_cols):
    """Build the SPMD bass program.

    n_tiles: number of 128-row tiles per core (must be divisible by 8).
    tile_col: list, local bag-column index for each tile (same on all cores).
    n_cols: number of local bag columns.
    """
    import concourse.bacc as bacc
    import concourse.mybir as mybir
    from concourse.tile import TileContext
    from concourse.masks import make_identity

    f32 = mybir.dt.float32
    bf16 = mybir.dt.bfloat16
    FC = F // P  # 4 feature chunks
    MC = HID // P  # 2 hidden chunks
    rows_per_core = n_tiles * P
    n_groups = n_tiles // JT
    n_blocks = rows_per_core // BLK
    GPB = BLK // GR  # groups per block
    LOOKAHEAD = 2  # blocks of DMA prefetch

    nc = bacc.Bacc(
        "TRN2", target_bir_lowering=False, debug=False, num_devices=N_CORES
    )

    x = nc.declare_dram_parameter("x", [rows_per_core, F], bf16, isOutput=False)
    xt = nc.declare_dram_parameter("xt", [F, rows_per_core], bf16, isOutput=False)
    w1 = nc.declare_dram_parameter("w1", [P, FC, MC, P], bf16, isOutput=False)
    b1 = nc.declare_dram_parameter("b1", [P, MC], f32, isOutput=False)
    w2 = nc.declare_dram_parameter("w2", [P, MC, 1], bf16, isOutput=False)
    u_out = nc.declare_dram_parameter("u", [P, FC, n_cols], f32, isOutput=True)
    w_out = nc.declare_dram_parameter("w", [P, n_tiles], bf16, isOutput=True)

    with TileContext(nc) as tc:
        with (
            tc.tile_pool(name="const", bufs=1) as const_pool,
            tc.tile_pool(name="xb", bufs=4) as xb_pool,
            tc.tile_pool(name="xt", bufs=3) as xt_pool,
            tc.tile_pool(name="xte", bufs=8) as xte_pool,
            tc.tile_pool(name="th", bufs=3) as th_pool,
            tc.tile_pool(name="out_sb", bufs=1) as out_pool,
            tc.tile_pool(name="hp", bufs=3, space="PSUM") as hp_pool,
            tc.tile_pool(name="sp", bufs=2, space="PSUM") as sp_pool,
            tc.tile_pool(name="acc", bufs=1, space="PSUM") as acc_pool,
            tc.tile_pool(name="pt", bufs=2, space="PSUM") as pt_pool,
        ):
            xb_hist = {}
            xt_hist = {}
            xt_parts = {}
            # this block's X^T is built on-device by PE transposes of its
            # xb tile (its 2MB xt DMA is skipped): trades idle PE/DVE for
            # DMA stream time on a DMA-bound kernel.
            tr_blocks = {1, 2} if n_blocks > 4 else set()
            tr_groups = {
                g
                for tb in tr_blocks
                for g in range(tb * GPB, (tb + 1) * GPB)
            }

            def emit_load(bb, split_xt=False):
                # X^T block first (it gates the H matmuls): partition p
                # holds feature rows {128c + p}, 1024 columns (2KB x 4
                # runs per partition).  Block 0 is loaded as two halves
                # so the first H matmuls start sooner.
                if bb in tr_blocks:
                    pass  # X^T comes from emit_transpose_group
                elif split_xt:
                    for hh in range(GPB):
                        xth = xte_pool.tile(
                            [P, FC, GR], bf16, name="xth", tag="xth"
                        )
                        xt_parts[bb * GPB + hh] = xth
                        nc.sync.dma_start(
                            out=xth,
                            in_=xt[
                                :, bb * BLK + hh * GR : bb * BLK + (hh + 1) * GR
                            ].rearrange("(c p) i -> p c i", p=P),
                        )
                else:
                    xtt = xt_pool.tile([P, FC, BLK], bf16, name="xt", tag="xt")
                    xt_hist[bb] = xtt
                    nc.sync.dma_start(
                        out=xtt,
                        in_=xt[:, bb * BLK : (bb + 1) * BLK].rearrange(
                            "(c p) i -> p c i", p=P
                        ),
                    )
                # partition p holds rows {8p+q} of the block: 8KB
                # contiguous per partition in bf16.
                xbt = xb_pool.tile([P, BLK // P, F], bf16, name="xb", tag="xb")
                xb_hist[bb] = xbt
                nc.sync.dma_start(
                    out=xbt,
                    in_=x[bb * BLK : (bb + 1) * BLK, :].rearrange(
                        "(p q) f -> p q f", p=P
                    ),
                )

            # ---- constants (host pre-chunked, no on-device relayout) ----
            # w1b[p, c, m, j] = W1[c*128+p, m*128+j], bf16
            w1b = const_pool.tile([P, FC, MC, P], bf16)
            nc.sync.dma_start(out=w1b, in_=w1[:, :, :, :])
            # w2b[p, m, 0] = W2[m*128+p, 0], bf16
            w2b = const_pool.tile([P, MC, 1], bf16)
            nc.sync.dma_start(out=w2b, in_=w2[:, :, :])
            # b1s[p, m] = b1[m*128+p], fp32 (per-partition tanh bias)
            b1s = const_pool.tile([P, MC], f32)
            nc.sync.dma_start(out=b1s, in_=b1[:, :])

            emit_load(0, split_xt=True)
            for bb in range(1, min(LOOKAHEAD + 1, n_blocks)):
                emit_load(bb, split_xt=(bb == n_blocks - 1))

            ident = const_pool.tile([P, P], bf16)
            make_identity(nc, ident)

            def emit_transpose_group(gg):
                # X^T tile for one 512-row group from xb chunks: 16 PE
                # transposes -> PSUM (2 halves), 4 DVE copies -> SBUF,
                # emitted 2 groups ahead of its H matmuls.
                tb, th_ = divmod(gg, GPB)
                xb_g = xb_hist[tb]
                xth = xte_pool.tile([P, FC, GR], bf16, name="xth", tag="xth")
                xt_parts[gg] = xth
                for half in range(2):
                    pt = pt_pool.tile([P, 2, FC, P], bf16, name="pt", tag="pt")
                    for jj in range(2):
                        q = JT * th_ + 2 * half + jj
                        for c in range(FC):
                            nc.tensor.matmul(
                                pt[:, jj, c, :],
                                xb_g[:, q, c * P : (c + 1) * P],
                                ident,
                                is_transpose=True,
                                start=(jj == 0 and c == 0),
                                stop=(jj == 1 and c == FC - 1),
                            )
                    for jj in range(2):
                        j = 2 * half + jj
                        nc.vector.tensor_copy(
                            out=xth[:, :, j * P : (j + 1) * P],
                            in_=pt[:, jj],
                        )

            # softmax weights for every row, bf16 (also read back by the
            # host to form the global softmax denominator)
            wsave = const_pool.tile([P, n_tiles], bf16)

            # U^T accumulators: one [128, FC] PSUM tile per bag while its
            # 16 tiles accumulate; copied into u_sb at bag end.
            u_sb = out_pool.tile([P, FC, n_cols], f32)
            first_tile = {}
            last_tile = {}
            for t, cl in enumerate(tile_col):
                first_tile.setdefault(cl, t)
                last_tile[cl] = t
            u_bag = [None] * n_cols

            th_hist = {}
            sp_hist = {}

            def emit_s(gg):
                th_g = th_hist[gg]
                sp = sp_pool.tile([P, JT], f32, name="sp", tag="sp")
                sp_hist[gg] = sp
                for j in range(JT):
                    for m in range(MC):
                        nc.tensor.matmul(
                            sp[:, j : j + 1],
                            th_g[:, m, j, :],
                            w2b[:, m, :],
                            start=(j == 0 and m == 0),
                            stop=(j == JT - 1 and m == MC - 1),
                        )
                nc.scalar.activation(
                    wsave[:, gg * JT : (gg + 1) * JT],
                    sp,
                    mybir.ActivationFunctionType.Exp,
                )
                del sp_hist[gg]
                del th_hist[gg]

            def emit_u(gg):
                bb, h = divmod(gg, GPB)
                xb_g = xb_hist[bb]
                for j in range(JT):
                    t = JT * gg + j
                    q = JT * h + j
                    col = tile_col[t]
                    if u_bag[col] is None:
                        u_bag[col] = acc_pool.tile(
                            [P, FC], f32, name="u_bag", tag="u_bag"
                        )
                    for c in range(FC):
                        # start=True marks the whole 2KB zero region
                        # pending-zero, so only the bag's first matmul
                        # starts; c=1..3 of the first tile then write
                        # (not accumulate) their still-pending columns.
                        nc.tensor.matmul(
                            u_bag[col][:, c : c + 1],
                            xb_g[:, q, c * P : (c + 1) * P],
                            wsave[:, t : t + 1],
                            start=(t == first_tile[col] and c == 0),
                            stop=(t == last_tile[col] and c == FC - 1),
                        )
                    if t == last_tile[col]:
                        nc.vector.tensor_copy(out=u_sb[:, :, col], in_=u_bag[col])
                        u_bag[col] = None
                if h == GPB - 1:
                    del xb_hist[bb]

            # ---- main loop over 512-row groups (software-pipelined) ----
            for g in range(n_groups):
                bb, h = divmod(g, GPB)
                if h == 0 and (bb + LOOKAHEAD) < n_blocks and (
                    bb + LOOKAHEAD
                ) not in xb_hist:
                    emit_load(
                        bb + LOOKAHEAD,
                        split_xt=(bb + LOOKAHEAD == n_blocks - 1),
                    )
                edge = g in xt_parts
                xtb = xt_parts[g] if edge else xt_hist[bb]

                # H^T[m*128+p, r] over the group's 512 rows, accumulating
                # feature chunks; then tanh(H^T + b1) -> bf16 per m-chunk.
                # c-inner (same-PSUM-bank consecutive) order: measured
                # faster than m-alternating, whose bank switch exposes the
                # next stationary load.
                th = th_pool.tile([P, MC, JT, P], bf16)
                th_hist[g] = th
                for m in range(MC):
                    hp = hp_pool.tile([P, GR], f32, name="hp", tag="hp")
                    for c in range(FC):
                        nc.tensor.matmul(
                            hp,
                            w1b[:, c, m, :],
                            (xtb[:, c, :] if edge else
                             xtb[:, c, h * GR : (h + 1) * GR]),
                            start=(c == 0),
                            stop=(c == FC - 1),
                        )
                    nc.scalar.activation(
                        th[:, m],
                        hp.rearrange("p (j r) -> p j r", j=JT),
                        mybir.ActivationFunctionType.Tanh,
                        bias=b1s[:, m : m + 1],
                    )
                if edge:
                    del xt_parts[g]
                elif h == GPB - 1:
                    del xt_hist[bb]
                if (g + 2) in tr_groups:
                    emit_transpose_group(g + 2)

                # pipelined: scores for g-1, bag accumulation for g-2
                if g >= 1:
                    emit_s(g - 1)
                if g >= 2:
                    emit_u(g - 2)

            # drain: scores for the last group first so its exp (ACT)
            # overlaps the U matmuls of g-2 on PE
            emit_s(n_groups - 1)
            emit_u(n_groups - 2)
            # exp weights are complete: stream them out under the last Us,
            # along with every bag but the last (already copied to SBUF)
            nc.sync.dma_start(out=w_out[:, :], in_=wsave)
            if n_cols > 1:
                nc.sync.dma_start(
                    out=u_out[:, :, : n_cols - 1], in_=u_sb[:, :, : n_cols - 1]
                )
            emit_u(n_groups - 1)

            # ---- epilogue: DMA the last bag's U^T out ----
            nc.sync.dma_start(
                out=u_out[:, :, n_cols - 1 :], in_=u_sb[:, :, n_cols - 1 :]
            )

    nc.compile()
    return nc


def _host_prep(X_core):
    """Cast the core's X shard to bf16 and build the permuted X^T copy.

    xb: natural [rows, F] bf16 (the device DMA applies the {8p+q} row
        permutation via its access pattern).
    xt: [F, rows] bf16 with columns permuted so that block bb's columns
        are ordered (q, p) -> row bb*1024 + 8p + q, matching xb's layout.
    """
    rows = X_core.shape[0]
    xb = np.ascontiguousarray(X_core).astype(ml_dtypes.bfloat16)
    xt = (
        X_core.reshape(rows // BLK, P, BLK // P, F)
        .transpose(3, 0, 2, 1)
        .reshape(F, rows)
        .astype(ml_dtypes.bfloat16)
    )
    return xb, np.ascontiguousarray(xt)


def _run_device(X, W1, b1, W2, bag_rows, trace=False, trace_kwargs=None):
    from concourse.bass_utils import run_bass_kernel_spmd

    rows_per_core = X.shape[0] // N_CORES
    n_tiles = rows_per_core // P
    tiles_per_bag = bag_rows // P
    n_cols = n_tiles // tiles_per_bag
    tile_col = [t // tiles_per_bag for t in range(n_tiles)]

    key = (rows_per_core, bag_rows)
    if key in _COMPILED_CACHE:
        nc = _COMPILED_CACHE[key]
    else:
        nc = _build_program(n_tiles, tile_col, n_cols)
        _COMPILED_CACHE[key] = nc

    FC = F // P
    MC = HID // P
    w1b = np.ascontiguousarray(
        np.asarray(W1, np.float32).reshape(FC, P, MC, P).transpose(1, 0, 2, 3)
    ).astype(ml_dtypes.bfloat16)
    w2b = np.ascontiguousarray(
        np.asarray(W2, np.float32).reshape(MC, P, 1).transpose(1, 0, 2)
    ).astype(ml_dtypes.bfloat16)
    b1s = np.ascontiguousarray(
        np.asarray(b1, np.float32).reshape(MC, P).T, np.float32
    )

    in_maps = []
    for c in range(N_CORES):
        xb_c, xt_c = _host_prep(
            np.asarray(
                X[c * rows_per_core : (c + 1) * rows_per_core], np.float32
            )
        )
        in_maps.append(
            {"x": xb_c, "xt": xt_c, "w1": w1b, "b1": b1s, "w2": w2b}
        )
    kw = dict(trace_kwargs or {})
    res = run_bass_kernel_spmd(
        nc, in_maps, list(range(N_CORES)), trace=trace, **kw
    )

    U = np.zeros((N_CORES * n_cols, F), np.float32)
    Z = np.float64(0.0)
    for c in range(N_CORES):
        # u[p, fc, col] -> U[col, fc*128 + p]
        U[c * n_cols : (c + 1) * n_cols] = (
            np.asarray(res.results[c]["u"]).transpose(2, 1, 0).reshape(n_cols, F)
        )
        Z += np.asarray(res.results[c]["w"], np.float64).sum()
    return U, Z, res


def _kernel_numpy(instance_features, bag_sizes, W1, b1, W2, b2):
    """Exact-math fallback for bag layouts the device program doesn't cover."""
    X = np.asarray(instance_features, np.float32)
    s = np.tanh(X @ W1 + b1) @ W2.reshape(-1, 1) + np.asarray(b2).reshape(1, -1)
    s = s - s.max()
    w = np.exp(s)
    w = w / w.sum()
    offsets = np.cumsum(np.asarray(bag_sizes, np.int64))
    seg = np.searchsorted(offsets, np.arange(X.shape[0]), side="right")
    out = np.zeros((len(bag_sizes), X.shape[1]), np.float32)
    np.add.at(out, seg[seg < len(bag_sizes)], (X * w)[seg < len(bag_sizes)])
    return out


def kernel(**inputs):
    X = np.asarray(inputs["instance_features"], np.float32)
    bag_sizes = np.asarray(inputs["bag_sizes"], np.int64)
    W1 = np.asarray(inputs["W1"], np.float32)
    b1 = np.asarray(inputs["b1"], np.float32)
    W2 = np.asarray(inputs["W2"], np.float32)
    b2 = np.asarray(inputs["b2"], np.float32)

    T, Fdim = X.shape
    B = bag_sizes.shape[0]
    bag = int(bag_sizes[0]) if B else 0
    # Device path constraints: equal whole bags per core, 1024-row DMA
    # blocks, and the row permutation needs bag_rows % 1024 == 0.
    aligned = (
        Fdim == F
        and B > 0
        and np.all(bag_sizes == bag)
        and bag % BLK == 0
        and bag * B == T
        and T % N_CORES == 0
        and (T // N_CORES) % BLK == 0
        and (T // N_CORES) % bag == 0
    )
    if not aligned:
        return _kernel_numpy(X, bag_sizes, W1, b1, W2, b2)

    U, Z, _ = _run_device(X, W1, b1, W2, bag)
    return (U / np.float32(Z)).astype(np.float32)


# revision 32
# speedup vs baseline: 1.2494x; 1.0038x over previous
"""AttentionMILPooling Trainium2 kernel.

Math (matches the jax reference):
    scores  = tanh(X @ W1 + b1) @ W2 + b2          # [T, 1]
    weights = softmax(scores, axis=0)              # global over all T
    out[b]  = sum_{i in bag b} weights[i] * X[i]   # [64, 512]

Key identities used:
  * b2 cancels exactly in the softmax, so it is dropped.
  * scores are bounded (|s| <= sum|W2| ~ 13) so no max-subtraction is
    needed; exp stays well inside fp32 range.
  * out[b] = (sum_{i in b} exp(s_i) * X_i) / Z with Z = sum_i exp(s_i):
    each core computes unnormalized per-bag sums U and returns all its
    per-row exp values; the host sums Z globally and divides once.

Structure: the host pre-casts X to bf16 and ALSO passes a pre-transposed
copy X^T (total DRAM traffic unchanged vs one fp32 copy), which removes
the entire PE transpose pass and all PSUM->SBUF relayout copies of the
earlier design.  The per-bag weighted sums are computed with X tiles as
the *stationary* operand and the exp-weight column as a 1-column moving
operand (output in [feature, bag] layout, untransposed on the host), so
their PE cost is stationary-load-bound instead of 512-column streams.
The kernel is DMA-bound (all 16 queues ~100% busy streaming 33.6MB/core
at ~392GB/s aggregate).  DMA blocks are 2048 rows -- the largest that
keeps a whole block inside one bag -- giving 16KB (xb) / 4KB (xt)
contiguous runs per partition, which measured ~13% faster per queue than
1024-row blocks.  The first and last blocks' X^T arrive as four
independent per-group quarter-tiles so the pipeline ramps as soon as
512KB lands and the final PE chunk after the stream drains is small;
outputs stream out under the drain.  Blocks 1-2 skip their X^T DMA
entirely: their X^T tiles are rebuilt on-device by PE transposes of the
already-loaded x tiles (+4.2us idle-PE work per block buys 5.1us of
saturated stream time each) -- the measured PE/DMA balance point.
Measured: 111.4-112.0us HW exec (156.3us session baseline).

Row permutation: within each 2048-row block, partition p holds rows
{16p+q}. X^T is passed with columns in the same permuted order
(host-side permute), so scores, exp-weights and U tiles all index rows
identically.  All math is row-order-free within a bag (bag_rows % 2048
== 0 keeps whole blocks inside one bag).

Per-core device pipeline, per 512-row group g (32 groups; block = 4
groups, DMA'd as one 2MB xb load [128,16,512] + one 2MB xt load
[128,4,2048]):
  PE  : 8x matmul H^T = W1^T @ X^T (accum 4 f-chunks, 2 hid-chunks)
  ACT : 2x tanh(H^T + b1) -> bf16 (b1 as per-partition bias)
  PE  : 8x matmul s = tanhH^T.T @ W2 (1-col moving, accum hid-chunks)
  ACT : 1x exp(s) -> wsave column (bf16)
  PE  : 16x matmul U^T[bag] += X_chunk^T @ w_col (X stationary, 1-col)
Emission is software-pipelined (s for g-1, U for g-2) so the in-order PE
stream never waits on same-group ACT results.  Per-bag U^T accumulates
in a [128, 4] PSUM tile over the bag's 16 tiles (start=True on the first
matmul marks the whole 2KB zero region pending-zero, so the other
columns write clean), then is copied to SBUF and DMA'd out at the end
([feature, bag] layout; host transposes).
"""

import numpy as np
import ml_dtypes

N_CORES = 8
F = 512  # feature dim
HID = 256  # hidden dim
P = 128  # partitions
BLK = 2048  # rows per DMA block
JT = 4  # 128-row tiles per processing group
GR = JT * P  # rows per processing group (512)

_COMPILED_CACHE = {}


def _build_program(n_tiles, tile_col, n_cols):
    """Build the SPMD bass program.

    n_tiles: number of 128-row tiles per core (must be divisible by 8).
    tile_col: list, local bag-column index for each tile (same on all cores).
    n_cols: number of local bag columns.
    """
    import concourse.bacc as bacc
    import concourse.mybir as mybir
    from concourse.tile import TileContext
    from concourse.masks import make_identity

    f32 = mybir.dt.float32
    bf16 = mybir.dt.bfloat16
    FC = F // P  # 4 feature chunks
    MC = HID // P  # 2 hidden chunks
    rows_per_core = n_tiles * P
    n_groups = n_tiles // JT
    n_blocks = rows_per_core // BLK
    GPB = BLK // GR  # groups per block
    LOOKAHEAD = 2  # blocks of DMA prefetch

    nc = bacc.Bacc(
        "TRN2", target_bir_lowering=False, debug=False, num_devices=N_CORES
    )

    x = nc.declare_dram_parameter("x", [rows_per_core, F], bf16, isOutput=False)
    xt = nc.declare_dram_parameter("xt", [F, rows_per_core], bf16, isOutput=False)
    w1 = nc.declare_dram_parameter("w1", [P, FC, MC, P], bf16, isOutput=False)
    b1 = nc.declare_dram_parameter("b1", [P, MC], f32, isOutput=False)
    w2 = nc.declare_dram_parameter("w2", [P, MC, 1], bf16, isOutput=False)
    u_out = nc.declare_dram_parameter("u", [P, FC, n_cols], f32, isOutput=True)
    w_out = nc.declare_dram_parameter("w", [P, n_tiles], bf16, isOutput=True)

    with TileContext(nc) as tc:
        with (
            tc.tile_pool(name="const", bufs=1) as const_pool,
            tc.tile_pool(name="xb", bufs=4) as xb_pool,
            tc.tile_pool(name="xt", bufs=3) as xt_pool,
            tc.tile_pool(name="xte", bufs=8) as xte_pool,
            tc.tile_pool(name="th", bufs=3) as th_pool,
            tc.tile_pool(name="out_sb", bufs=1) as out_pool,
            tc.tile_pool(name="hp", bufs=4, space="PSUM") as hp_pool,
            tc.tile_pool(name="sp", bufs=1, space="PSUM") as sp_pool,
            tc.tile_pool(name="acc", bufs=1, space="PSUM") as acc_pool,
            tc.tile_pool(name="pt", bufs=2, space="PSUM") as pt_pool,
        ):
            xb_hist = {}
            xt_hist = {}
            xt_parts = {}
            # this block's X^T is built on-device by PE transposes of its
            # xb tile (its 2MB xt DMA is skipped): trades idle PE/DVE for
            # DMA stream time on a DMA-bound kernel.
            tr_blocks = {1, 2} if n_blocks > 4 else set()
            tr_groups = {
                g
                for tb in tr_blocks
                for g in range(tb * GPB, (tb + 1) * GPB)
            }

            def emit_load(bb, split_xt=False):
                # X^T block first (it gates the H matmuls): partition p
                # holds feature rows {128c + p}, 1024 columns (2KB x 4
                # runs per partition).  Block 0 is loaded as two halves
                # so the first H matmuls start sooner.
                if bb in tr_blocks:
                    pass  # X^T comes from emit_transpose_group
                elif split_xt:
                    for hh in range(GPB):
                        xth = xte_pool.tile(
                            [P, FC, GR], bf16, name="xth", tag="xth"
                        )
                        xt_parts[bb * GPB + hh] = xth
                        nc.sync.dma_start(
                            out=xth,
                            in_=xt[
                                :, bb * BLK + hh * GR : bb * BLK + (hh + 1) * GR
                            ].rearrange("(c p) i -> p c i", p=P),
                        )
                else:
                    xtt = xt_pool.tile([P, FC, BLK], bf16, name="xt", tag="xt")
                    xt_hist[bb] = xtt
                    nc.sync.dma_start(
                        out=xtt,
                        in_=xt[:, bb * BLK : (bb + 1) * BLK].rearrange(
                            "(c p) i -> p c i", p=P
                        ),
                    )
                # partition p holds rows {8p+q} of the block: 8KB
                # contiguous per partition in bf16.
                xbt = xb_pool.tile([P, BLK // P, F], bf16, name="xb", tag="xb")
                xb_hist[bb] = xbt
                nc.sync.dma_start(
                    out=xbt,
                    in_=x[bb * BLK : (bb + 1) * BLK, :].rearrange(
                        "(p q) f -> p q f", p=P
                    ),
                )

            # ---- constants (host pre-chunked, no on-device relayout) ----
            # w1b[p, c, m, j] = W1[c*128+p, m*128+j], bf16
            w1b = const_pool.tile([P, FC, MC, P], bf16)
            nc.sync.dma_start(out=w1b, in_=w1[:, :, :, :])
            # w2b[p, m, 0] = W2[m*128+p, 0], bf16
            w2b = const_pool.tile([P, MC, 1], bf16)
            nc.sync.dma_start(out=w2b, in_=w2[:, :, :])
            # b1s[p, m] = b1[m*128+p], fp32 (per-partition tanh bias)
            b1s = const_pool.tile([P, MC], f32)
            nc.sync.dma_start(out=b1s, in_=b1[:, :])

            emit_load(0, split_xt=True)
            for bb in range(1, min(LOOKAHEAD + 1, n_blocks)):
                emit_load(bb, split_xt=(bb == n_blocks - 1))

            ident = const_pool.tile([P, P], bf16)
            make_identity(nc, ident)

            def emit_transpose_group(gg):
                # X^T tile for one 512-row group from xb chunks: 16 PE
                # transposes -> PSUM (2 halves), 4 DVE copies -> SBUF,
                # emitted 2 groups ahead of its H matmuls.
                tb, th_ = divmod(gg, GPB)
                xb_g = xb_hist[tb]
                xth = xte_pool.tile([P, FC, GR], bf16, name="xth", tag="xth")
                xt_parts[gg] = xth
                for half in range(2):
                    pt = pt_pool.tile([P, 2, FC, P], bf16, name="pt", tag="pt")
                    for jj in range(2):
                        q = JT * th_ + 2 * half + jj
                        for c in range(FC):
                            nc.tensor.matmul(
                                pt[:, jj, c, :],
                                xb_g[:, q, c * P : (c + 1) * P],
                                ident,
                                is_transpose=True,
                                start=(jj == 0 and c == 0),
                                stop=(jj == 1 and c == FC - 1),
                            )
                    for jj in range(2):
                        j = 2 * half + jj
                        nc.vector.tensor_copy(
                            out=xth[:, :, j * P : (j + 1) * P],
                            in_=pt[:, jj],
                        )

            # softmax weights for every row, bf16 (also read back by the
            # host to form the global softmax denominator)
            wsave = const_pool.tile([P, n_tiles], bf16)

            # U^T accumulators: one [128, FC] PSUM tile per bag while its
            # 16 tiles accumulate; copied into u_sb at bag end.
            u_sb = out_pool.tile([P, FC, n_cols], f32)
            first_tile = {}
            last_tile = {}
            for t, cl in enumerate(tile_col):
                first_tile.setdefault(cl, t)
                last_tile[cl] = t
            u_bag = [None] * n_cols

            th_hist = {}
            sp_hist = {}

            def emit_s(gg):
                th_g = th_hist[gg]
                sp = sp_pool.tile([P, JT], f32, name="sp", tag="sp")
                sp_hist[gg] = sp
                for j in range(JT):
                    for m in range(MC):
                        nc.tensor.matmul(
                            sp[:, j : j + 1],
                            th_g[:, m, j, :],
                            w2b[:, m, :],
                            start=(j == 0 and m == 0),
                            stop=(j == JT - 1 and m == MC - 1),
                        )
                nc.scalar.activation(
                    wsave[:, gg * JT : (gg + 1) * JT],
                    sp,
                    mybir.ActivationFunctionType.Exp,
                )
                del sp_hist[gg]
                del th_hist[gg]

            def emit_u(gg):
                bb, h = divmod(gg, GPB)
                xb_g = xb_hist[bb]
                for j in range(JT):
                    t = JT * gg + j
                    q = JT * h + j
                    col = tile_col[t]
                    if u_bag[col] is None:
                        u_bag[col] = acc_pool.tile(
                            [P, FC], f32, name="u_bag", tag="u_bag"
                        )
                    for c in range(FC):
                        # start=True marks the whole 2KB zero region
                        # pending-zero, so only the bag's first matmul
                        # starts; c=1..3 of the first tile then write
                        # (not accumulate) their still-pending columns.
                        nc.tensor.matmul(
                            u_bag[col][:, c : c + 1],
                            xb_g[:, q, c * P : (c + 1) * P],
                            wsave[:, t : t + 1],
                            start=(t == first_tile[col] and c == 0),
                            stop=(t == last_tile[col] and c == FC - 1),
                        )
                    if t == last_tile[col]:
                        nc.vector.tensor_copy(out=u_sb[:, :, col], in_=u_bag[col])
                        u_bag[col] = None
                if h == GPB - 1:
                    del xb_hist[bb]

            # ---- main loop over 512-row groups (software-pipelined) ----
            for g in range(n_groups):
                bb, h = divmod(g, GPB)
                if h == 0 and (bb + LOOKAHEAD) < n_blocks and (
                    bb + LOOKAHEAD
                ) not in xb_hist:
                    emit_load(
                        bb + LOOKAHEAD,
                        split_xt=(bb + LOOKAHEAD == n_blocks - 1),
                    )
                edge = g in xt_parts
                xtb = xt_parts[g] if edge else xt_hist[bb]

                # H^T[m*128+p, r] over the group's 512 rows, accumulating
                # feature chunks; then tanh(H^T + b1) -> bf16 per m-chunk.
                # c-inner (same-PSUM-bank consecutive) order: measured
                # faster than m-alternating, whose bank switch exposes the
                # next stationary load.
                th = th_pool.tile([P, MC, JT, P], bf16)
                th_hist[g] = th
                for m in range(MC):
                    hp = hp_pool.tile([P, GR], f32, name="hp", tag="hp")
                    for c in range(FC):
                        nc.tensor.matmul(
                            hp,
                            w1b[:, c, m, :],
                            (xtb[:, c, :] if edge else
                             xtb[:, c, h * GR : (h + 1) * GR]),
                            start=(c == 0),
                            stop=(c == FC - 1),
                        )
                    nc.scalar.activation(
                        th[:, m],
                        hp.rearrange("p (j r) -> p j r", j=JT),
                        mybir.ActivationFunctionType.Tanh,
                        bias=b1s[:, m : m + 1],
                    )
                if edge:
                    del xt_parts[g]
                elif h == GPB - 1:
                    del xt_hist[bb]
                if (g + 2) in tr_groups:
                    emit_transpose_group(g + 2)

                # pipelined: scores for g-1, bag accumulation for g-2
                if g >= 1:
                    emit_s(g - 1)
                if g >= 2:
                    emit_u(g - 2)

            # drain: scores for the last group first so its exp (ACT)
            # overlaps the U matmuls of g-2 on PE
            emit_s(n_groups - 1)
            emit_u(n_groups - 2)
            # exp weights are complete: stream them out under the last Us,
            # along with every bag but the last (already copied to SBUF)
            nc.sync.dma_start(out=w_out[:, :], in_=wsave)
            if n_cols > 1:
                nc.sync.dma_start(
                    out=u_out[:, :, : n_cols - 1], in_=u_sb[:, :, : n_cols - 1]
                )
            emit_u(n_groups - 1)

            # ---- epilogue: DMA the last bag's U^T out ----
            nc.sync.dma_start(
                out=u_out[:, :, n_cols - 1 :], in_=u_sb[:, :, n_cols - 1 :]
            )

    nc.compile()
    return nc


def _host_prep(X_core):
    """Cast the core's X shard to bf16 and build the permuted X^T copy.

    xb: natural [rows, F] bf16 (the device DMA applies the {8p+q} row
        permutation via its access pattern).
    xt: [F, rows] bf16 with columns permuted so that block bb's columns
        are ordered (q, p) -> row bb*1024 + 8p + q, matching xb's layout.
    """
    rows = X_core.shape[0]
    xb = np.ascontiguousarray(X_core).astype(ml_dtypes.bfloat16)
    xt = (
        X_core.reshape(rows // BLK, P, BLK // P, F)
        .transpose(3, 0, 2, 1)
        .reshape(F, rows)
        .astype(ml_dtypes.bfloat16)
    )
    return xb, np.ascontiguousarray(xt)


def _run_device(X, W1, b1, W2, bag_rows, trace=False, trace_kwargs=None):
    from concourse.bass_utils import run_bass_kernel_spmd

    rows_per_core = X.shape[0] // N_CORES
    n_tiles = rows_per_core // P
    tiles_per_bag = bag_rows // P
    n_cols = n_tiles // tiles_per_bag
    tile_col = [t // tiles_per_bag for t in range(n_tiles)]

    key = (rows_per_core, bag_rows)
    if key in _COMPILED_CACHE:
        nc = _COMPILED_CACHE[key]
    else:
        nc = _build_program(n_tiles, tile_col, n_cols)
        _COMPILED_CACHE[key] = nc

    FC = F // P
    MC = HID // P
    w1b = np.ascontiguousarray(
        np.asarray(W1, np.float32).reshape(FC, P, MC, P).transpose(1, 0, 2, 3)
    ).astype(ml_dtypes.bfloat16)
    w2b = np.ascontiguousarray(
        np.asarray(W2, np.float32).reshape(MC, P, 1).transpose(1, 0, 2)
    ).astype(ml_dtypes.bfloat16)
    b1s = np.ascontiguousarray(
        np.asarray(b1, np.float32).reshape(MC, P).T, np.float32
    )

    in_maps = []
    for c in range(N_CORES):
        xb_c, xt_c = _host_prep(
            np.asarray(
                X[c * rows_per_core : (c + 1) * rows_per_core], np.float32
            )
        )
        in_maps.append(
            {"x": xb_c, "xt": xt_c, "w1": w1b, "b1": b1s, "w2": w2b}
        )
    kw = dict(trace_kwargs or {})
    res = run_bass_kernel_spmd(
        nc, in_maps, list(range(N_CORES)), trace=trace, **kw
    )

    U = np.zeros((N_CORES * n_cols, F), np.float32)
    Z = np.float64(0.0)
    for c in range(N_CORES):
        # u[p, fc, col] -> U[col, fc*128 + p]
        U[c * n_cols : (c + 1) * n_cols] = (
            np.asarray(res.results[c]["u"]).transpose(2, 1, 0).reshape(n_cols, F)
        )
        Z += np.asarray(res.results[c]["w"], np.float64).sum()
    return U, Z, res


def _kernel_numpy(instance_features, bag_sizes, W1, b1, W2, b2):
    """Exact-math fallback for bag layouts the device program doesn't cover."""
    X = np.asarray(instance_features, np.float32)
    s = np.tanh(X @ W1 + b1) @ W2.reshape(-1, 1) + np.asarray(b2).reshape(1, -1)
    s = s - s.max()
    w = np.exp(s)
    w = w / w.sum()
    offsets = np.cumsum(np.asarray(bag_sizes, np.int64))
    seg = np.searchsorted(offsets, np.arange(X.shape[0]), side="right")
    out = np.zeros((len(bag_sizes), X.shape[1]), np.float32)
    np.add.at(out, seg[seg < len(bag_sizes)], (X * w)[seg < len(bag_sizes)])
    return out


def kernel(**inputs):
    X = np.asarray(inputs["instance_features"], np.float32)
    bag_sizes = np.asarray(inputs["bag_sizes"], np.int64)
    W1 = np.asarray(inputs["W1"], np.float32)
    b1 = np.asarray(inputs["b1"], np.float32)
    W2 = np.asarray(inputs["W2"], np.float32)
    b2 = np.asarray(inputs["b2"], np.float32)

    T, Fdim = X.shape
    B = bag_sizes.shape[0]
    bag = int(bag_sizes[0]) if B else 0
    # Device path constraints: equal whole bags per core, 1024-row DMA
    # blocks, and the row permutation needs bag_rows % 1024 == 0.
    aligned = (
        Fdim == F
        and B > 0
        and np.all(bag_sizes == bag)
        and bag % BLK == 0
        and bag * B == T
        and T % N_CORES == 0
        and (T // N_CORES) % BLK == 0
        and (T // N_CORES) % bag == 0
    )
    if not aligned:
        return _kernel_numpy(X, bag_sizes, W1, b1, W2, b2)

    U, Z, _ = _run_device(X, W1, b1, W2, bag)
    return (U / np.float32(Z)).astype(np.float32)
